# revision 13
# baseline (speedup 1.0000x reference)
"""Luong attention (dot-product attention with per-position scale) on 8 TRN2 cores.

Full-input contract: kernel(query[32,1024], values[32,4096,1024], scale[4096,1])
-> context[32,1024].  Batch is sharded 4-per-core across 8 NeuronCores
(data-parallel, no collectives).

Per-core plan (B=4 batches, S=4096, H=1024, fp32):
  - V[b] streamed HBM->SBUF once, tiles [128(s), 4(chunk), 1024(h)] (2 MiB DMAs).
  - scores[s] = sum_h V[s,h]*q[h] via fused DVE tensor_tensor_reduce
    (mult + free-axis add) against a partition-replicated copy of q.
    Layout: scores[p, c] with s = c*128 + p.
  - scores *= scale[s]  (scale pre-transposed to [128,32] once per core).
  - softmax: free-axis max (DVE) -> partition all-reduce max (GpSimd,
    replicated) -> Exp with fused row-sum on ScalarE -> global denominator
    Z via ones-matmul on PE -> fold 1/Z into the final context scale.
  - context = sum_s E[s] * V[s,:] natively on PE: E column [128,1] stationary,
    V tile [128,512] moving, PSUM-accumulated over 32 s-chunks.
V is read exactly once -> per-core HBM traffic ~64 MiB (memory roofline).
"""

import sys

sys.path.insert(0, "/opt/trn_rl_repo")

from contextlib import ExitStack

import numpy as np

import concourse.bacc as bacc
import concourse.bass as bass
import concourse.tile as tile
from concourse import bass_isa, mybir
from concourse.bass_utils import run_bass_kernel_spmd

F32 = mybir.dt.float32
BF16 = mybir.dt.bfloat16

N_CORES = 8
B_FULL = 32
S = 4096
H = 1024
B_PER_CORE = B_FULL // N_CORES  # 4

P = 128               # partitions
N_CHUNK = S // P      # 32 s-slots per partition; s = p*32 + j (partition-major)
CHUNKS_PER_DMA = 4    # 2 MiB per dma_start, 16 KiB contiguous per partition
N_DMA_GROUPS = N_CHUNK // CHUNKS_PER_DMA  # 8
VBUFS = 4             # fp32 staging slots (16 KiB/partition; freed after scores+cast)
BBUFS = 11            # bf16 V slots (8 KiB/partition; live until weighted sum)


def build_kernel(nb=B_PER_CORE, n_chunk=N_CHUNK, vbufs=VBUFS, bbufs=BBUFS):
    s = n_chunk * P
    nc = bacc.Bacc("TRN2", target_bir_lowering=False, debug=False)

    q_d = nc.dram_tensor("query", (nb, H), F32, kind="ExternalInput")
    v_d = nc.dram_tensor("values", (nb, s, H), F32, kind="ExternalInput")
    scale_d = nc.dram_tensor("scale", (s, 1), F32, kind="ExternalInput")
    out_d = nc.dram_tensor("out", (nb, H), F32, kind="ExternalOutput")

    n_groups = n_chunk // CHUNKS_PER_DMA

    with tile.TileContext(nc) as tc, ExitStack() as ctx:
        consts = ctx.enter_context(tc.tile_pool(name="consts", bufs=1))
        vpool = ctx.enter_context(tc.tile_pool(name="vpool", bufs=vbufs))
        bpool = ctx.enter_context(tc.tile_pool(name="bpool", bufs=bbufs))
        qpool = ctx.enter_context(tc.tile_pool(name="qpool", bufs=2))
        spool = ctx.enter_context(tc.tile_pool(name="spool", bufs=2))
        scratch = ctx.enter_context(tc.tile_pool(name="scratch", bufs=2))
        opool = ctx.enter_context(tc.tile_pool(name="opool", bufs=2))
        psum = ctx.enter_context(tc.tile_pool(name="psum", bufs=2, space="PSUM"))
        zpsum = ctx.enter_context(tc.tile_pool(name="zpsum", bufs=2, space="PSUM"))

        # ---- one-time constants ----
        ones_col = consts.tile([P, 1], F32)
        nc.vector.memset(ones_col, 1.0)

        # scale[s] -> scale_sb[p, j] with s = p*n_chunk + j (partition-major,
        # matching the V layout below) -- a direct strided DMA, no transpose.
        scale_sb = consts.tile([P, n_chunk], F32)
        nc.sync.dma_start(
            out=scale_sb[:],
            in_=scale_d.rearrange("(p j) o -> p (j o)", p=P),
        )

        for b in range(nb):
            # ---- replicate q[b] across all 128 partitions (DMA broadcast) ----
            q_rep = qpool.tile([P, H], F32)
            q_row = q_d[b]
            q_bcast = bass.AP(
                tensor=q_row.tensor,
                offset=q_row.offset,
                ap=[[0, P], q_row.ap[0]],
            )
            nc.gpsimd.dma_start(out=q_rep[:], in_=q_bcast)

            # ---- stream V[b]; fused dot-product per 128-position chunk ----
            v_view = v_d[b].rearrange("(p j) h -> p j h", p=P)
            vbtiles = []
            scores = spool.tile([P, n_chunk], F32)
            for g in range(n_groups):
                vt = vpool.tile([P, CHUNKS_PER_DMA, H], F32, tag="vt")
                nc.sync.dma_start(
                    out=vt[:],
                    in_=v_view[:, g * CHUNKS_PER_DMA : (g + 1) * CHUNKS_PER_DMA, :],
                )
                # bf16 copy for the weighted-sum matmuls (ScalarE is idle);
                # the fp32 staging tile frees once scores + cast are done.
                vb = bpool.tile([P, CHUNKS_PER_DMA, H], BF16, tag="vb")
                nc.scalar.copy(out=vb[:], in_=vt[:])
                vbtiles.append(vb)
                for cl in range(CHUNKS_PER_DMA):
                    c = g * CHUNKS_PER_DMA + cl
                    prod = scratch.tile([P, H], F32, tag="prod")
                    nc.vector.scalar_tensor_tensor(
                        out=prod[:],
                        in0=vt[:, cl, :],
                        scalar=1.0,
                        in1=q_rep[:],
                        op0=mybir.AluOpType.mult,
                        op1=mybir.AluOpType.mult,
                        accum_out=scores[:, c : c + 1],
                    )

            # ---- softmax over all s (partition x free) ----
            scores2 = spool.tile([P, n_chunk], F32, tag="scores2")
            nc.vector.tensor_mul(scores2[:], scores[:], scale_sb[:])

            m1 = spool.tile([P, 1], F32, tag="m1")
            nc.vector.tensor_reduce(
                out=m1[:], in_=scores2[:],
                axis=mybir.AxisListType.X, op=mybir.AluOpType.max,
            )
            m_all = spool.tile([P, 1], F32, tag="m_all")
            nc.gpsimd.partition_all_reduce(
                out_ap=m_all[:], in_ap=m1[:], channels=P,
                reduce_op=bass_isa.ReduceOp.max,
            )
            negm = spool.tile([P, 1], F32, tag="negm")
            nc.vector.tensor_scalar_mul(negm[:], m_all[:], -1.0)

            # E = exp(scores2 - m) in bf16 (matmul stationary); S1 = fp32 row
            # sums of the pre-rounding exp values (fused accumulate).
            e_t = spool.tile([P, n_chunk], BF16, tag="e_t")
            s1 = spool.tile([P, 1], F32, tag="s1")
            nc.scalar.activation(
                out=e_t[:], in_=scores2[:],
                func=mybir.ActivationFunctionType.Exp,
                bias=negm[:], scale=1.0,
                accum_out=s1[:],
            )

            # Z = sum_p S1[p] via ones-matmul -> [1,1] PSUM; r = 1/Z
            z_ps = zpsum.tile([1, 1], F32, tag="z")
            nc.tensor.matmul(z_ps[:], lhsT=s1[:], rhs=ones_col[:],
                             start=True, stop=True)
            r_sb = spool.tile([1, 1], F32, tag="r")
            nc.vector.reciprocal(out=r_sb[:], in_=z_ps[:])

            # ---- context = sum_c E[:,c]^T @ V_chunk  (PSUM-accumulated) ----
            ctx_ps = psum.tile([1, H], F32, tag="ctx")
            for c in range(n_chunk):
                vb = vbtiles[c // CHUNKS_PER_DMA]
                cl = c % CHUNKS_PER_DMA
                for h0 in range(0, H, 512):
                    nc.tensor.matmul(
                        ctx_ps[:, h0 : h0 + 512],
                        lhsT=e_t[:, c : c + 1],
                        rhs=vb[:, cl, h0 : h0 + 512],
                        start=(c == 0),
                        stop=(c == n_chunk - 1),
                    )

            ctx_sb = opool.tile([1, H], F32, tag="ctx_sb")
            nc.vector.tensor_scalar_mul(ctx_sb[:], ctx_ps[:], r_sb[:])
            nc.sync.dma_start(out=out_d[b : b + 1, :], in_=ctx_sb[:])

    nc.compile()
    return nc


_NC_CACHE = {}


def _get_nc():
    if "nc" not in _NC_CACHE:
        _NC_CACHE["nc"] = build_kernel()
    return _NC_CACHE["nc"]


def run(query, values, scale, trace=False, **kw):
    nc = _get_nc()
    query = np.ascontiguousarray(query, dtype=np.float32)
    values = np.ascontiguousarray(values, dtype=np.float32)
    scale = np.ascontiguousarray(scale, dtype=np.float32)
    in_maps = []
    for core in range(N_CORES):
        lo = core * B_PER_CORE
        hi = lo + B_PER_CORE
        in_maps.append(
            {"query": query[lo:hi], "values": values[lo:hi], "scale": scale}
        )
    res = run_bass_kernel_spmd(nc, in_maps, core_ids=list(range(N_CORES)),
                               trace=trace, **kw)
    out = np.concatenate([r["out"] for r in res.results], axis=0)
    return out, res


def kernel(query, values, scale):
    out, _ = run(query, values, scale)
    return out.astype(np.float32)
